# revision 21
# baseline (speedup 1.0000x reference)
"""Trainium2 Bass kernel for linear multi-head attention (v2).

Reference computation (B=4, S=8192, D=1024, H=16, DH=64):
    Q  = softmax((x@Wq) per-head over DH) * DH**-0.5
    K  = softmax((x@Wkv)[...,:DH] per-head over S)
    V  = (x@Wkv)[..., DH:]
    ctx = K^T @ V  per (b, h)               # [DH, DH]
    y  = (Q @ ctx  per head) @ Wlin + blin

Sharding v2: batch x seq-half. Core c handles batch b = c//2, sequence
half = c%2 -> 4096 rows per core, single local batch. The K-softmax
normalizer spans the full sequence, so each core accumulates the
unnormalized per-head context ctxU = sum_s exp(k_s) [v_s | 1] locally and
a PAIRWISE AllReduce (replica groups [2b, 2b+1]) sums the two halves --
4x less collective data and 4x fewer participants than seq-parallel x8.

Per-core pipeline (single pass over x; projection matmuls in bf16 with
fp32 accumulation):
  phase 1a (per 128-row tile): x tile -> cast bf16 -> PE-transpose ->
      xT kept RESIDENT in SBUF (8 MiB) -> KV = x@Wkv -> exp(K) ->
      per-head ctxU/Z single-shot matmuls accumulated IN PSUM across all
      32 tiles (per-element has_written handles first-write; only the
      first matmul per bank carries start=True)
  collective: pairwise AllReduce of [128, 8, 65] fp32 via DRAM bounce
  mid: ctxn = ctxU/Z * SCALE (bf16) -> PE-transpose -> M_j = ctxn_j @
      Wlin_j packed to M [128, 8, D] (bf16 or fp8).  Wlin is loaded
      fresh each rep here (it is only needed for 16 matmuls) to keep
      SBUF under budget; the DMA overlaps the collective.
  phase 1b (per tile, software-pipelined): Q = xT@Wq (resident xT, no
      reload/retranspose) -> exp -> row-normalize -> PE-transpose ->
      y = Qn @ M + blin -> store.  Stage A (Q matmul + exp) of the first
      HOIST tiles is emitted before the M-build so the PE never waits on
      the collective.
  The y matmul optionally runs in fp8-e4m3 DoubleRow (2x PE throughput):
      qn scaled by 2^6, M by 2^17, y unscaled by ACT copy with 2^-23.
  Wkv/Wq for rep i+1 are re-loaded during rep i's phase 1b so the rep
      boundary has no DMA stall (timing harness runs the body reps times).
"""

import sys

if "/opt/trn_rl_repo" not in sys.path:
    sys.path.insert(0, "/opt/trn_rl_repo")

from contextlib import ExitStack

import numpy as np

import concourse.bacc as bacc
import concourse.mybir as mybir
import concourse.tile as tile
from concourse.bass_utils import run_bass_kernel_spmd
from concourse.masks import make_identity

B, S, D = 4, 8192, 1024
H, DH = 16, 64
SCALE = DH ** -0.5
NCORES = 8
S_HALF = S // 2              # 4096 rows per core (one batch, half sequence)
ROWS = S_HALF
P = 128
NT = ROWS // P               # 32 tiles
NPAIR = H // 2               # 8 head pairs

F32 = mybir.dt.float32
BF16 = mybir.dt.bfloat16
FP8 = mybir.dt.float8e4
EXP = mybir.ActivationFunctionType.Exp
COPY = mybir.ActivationFunctionType.Copy
MUL = mybir.AluOpType.mult

Y_FP8 = True                 # y = Qn @ M in fp8-e4m3 DoubleRow
S1 = 64.0                    # qn scale (softmax values ~1/64)
S2 = float(2 ** 16)          # M scale (|M| ~ 3e-4, absmax ~2e-3; fp8e4
#                              max normal is 240 -> keep absmax*S2 < ~130)
HOIST = 16                   # phase-1b tiles run through qnT before M-build
NPRE = 3                     # next-rep x tiles prefetched during the tail


def _transpose_to(nc, psum_tp, dst, src, ident):
    """PE-transpose src [128, 1024] into dst [128, 8, 128] (feature-major)."""
    dt_ = src.dtype
    for g in range(2):
        tp = psum_tp.tile([P, 512], dt_, tag="tp")
        for k in range(4):
            c = g * 4 + k
            nc.tensor.transpose(tp[:, k * P:(k + 1) * P],
                                src[:, c * P:(c + 1) * P], ident)
        dst_v = dst[:, g * 4:(g + 1) * 4, :]
        if g == 0:
            nc.vector.tensor_copy(out=dst_v, in_=tp[:])
        else:
            nc.scalar.activation(dst_v, tp[:], COPY)


def _emit(tc, nc, x_d, wq_d, wkv_d, wlin_d, blin_d, bsel_d, y_d, reps,
          dbg=None,
          no_collective=False):
    with ExitStack() as top:
        const = top.enter_context(tc.tile_pool(name="const", bufs=1))
        dram = top.enter_context(tc.tile_pool(name="dram", bufs=1, space="DRAM"))
        # PSUM pools (8 banks exactly):
        #   psum_mm  [128,1024] x2 bufs = 4 banks: KV halves / Q / M-build
        #   psum_ctx [128,1024] x1 buf  = 2 banks: phase-1a ctx accumulator,
        #            reused as the y-matmul target in phase 1b
        #   psum_tp  [128,512]  x2 bufs = 2 banks: all PE transposes
        psum_mm = top.enter_context(tc.tile_pool(name="psum_mm", bufs=2, space="PSUM"))
        psum_ctx = top.enter_context(tc.tile_pool(name="psum_ctx", bufs=1, space="PSUM"))
        psum_tp = top.enter_context(tc.tile_pool(name="psum_tp", bufs=2, space="PSUM"))
        # long-lived SBUF: resident xT (8 MiB), Wkv/Wq (bf16), M
        resid = top.enter_context(tc.tile_pool(name="resid", bufs=1))
        work = top.enter_context(tc.tile_pool(name="work", bufs=2))
        xpool = top.enter_context(tc.tile_pool(name="xpool", bufs=3))

        ident_bf = const.tile([P, P], BF16, tag="ident_bf")
        make_identity(nc, ident_bf)
        blin_bc = const.tile([P, D], F32, tag="blin_bc")
        nc.sync.dma_start(blin_bc[:], blin_d[None, :].to_broadcast([P, D]))
        bsel_bc = const.tile([P, B], F32, tag="bsel_bc")
        nc.sync.dma_start(bsel_bc[:], bsel_d[None, :].to_broadcast([P, B]))
        zero_bf = const.tile([P, P], BF16, tag="zero_bf")
        nc.vector.memset(zero_bf[:], 0.0)

        wkv_sb = resid.tile([P, D // P, 2 * D], BF16, tag="wkv")
        wq_sb = resid.tile([P, D // P, D], BF16, tag="wq")
        xT_res = resid.tile([P, NT, D // P, P], BF16, tag="xTres")
        m_sb = resid.tile([P, NPAIR, D], FP8 if Y_FP8 else BF16, tag="M")

        wkv_src = wkv_d.rearrange("(c p) n -> p c n", p=P)
        wq_src = wq_d.rearrange("(c p) n -> p c n", p=P)
        wlin_src = wlin_d.rearrange("(c p) n -> p c n", p=P)

        def stage_cast(dst_slice, src_slice, use_act=False):
            wst = work.tile([P, 2, 512], F32, tag="wst", name="wst")
            nc.sync.dma_start(wst[:], src_slice)
            if use_act:
                nc.scalar.activation(dst_slice, wst[:], COPY)
            else:
                nc.vector.tensor_copy(out=dst_slice, in_=wst[:])

        def load_kvq():
            # wkv by (half, nb) column group so the first KV matmuls of the
            # next phase 1a wait on only 2 MiB of DMA
            for half in range(2):
                for nb in range(2):
                    col0 = half * D + nb * 512
                    for cc in range(0, 8, 2):
                        stage_cast(wkv_sb[:, cc:cc + 2, col0:col0 + 512],
                                   wkv_src[:, cc:cc + 2, col0:col0 + 512])
            for nb in range(2):
                for cc in range(0, 8, 2):
                    stage_cast(wq_sb[:, cc:cc + 2, nb * 512:(nb + 1) * 512],
                               wq_src[:, cc:cc + 2, nb * 512:(nb + 1) * 512])

        load_kvq()
        for rep in range(reps):
            _emit_once(tc, nc, x_d, y_d, dram,
                       (psum_mm, psum_ctx, psum_tp),
                       (wkv_sb, wq_sb, xT_res, m_sb),
                       wlin_src, stage_cast,
                       ident_bf, blin_bc, bsel_bc, zero_bf,
                       load_kvq if rep + 1 < reps else None,
                       xpool, prefetched=(rep > 0),
                       dbg=dbg, no_collective=no_collective)


def _emit_once(tc, nc, x_d, y_d, dram, psum, sbuf, wlin_src, stage_cast,
               ident_bf, blin_bc, bsel_bc, zero_bf, load_next_weights, xpool,
               prefetched=False, dbg=None, no_collective=False):
    psum_mm, psum_ctx, psum_tp = psum
    wkv_sb, wq_sb, xT_res, m_sb = sbuf
    cc_in = dram.tile([P, B, NPAIR, 65], F32, tag="cc_in")
    cc_out = dram.tile([P, B, NPAIR, 65], F32, tag="cc_out",
                       addr_space="Shared")

    # p2 outlives p1 (allocated first = disjoint zone); p1 closes after
    # phase 1a so the mid pools reuse its SBUF space.
    with ExitStack() as s1:
        p2 = s1.enter_context(tc.tile_pool(name="p2", bufs=2))

        def load_xT(t, pool, tag="xb"):
            x_nat = xpool.tile([P, D], F32, tag="x", name="x_nat")
            nc.sync.dma_start(x_nat[:], x_d[t * P:(t + 1) * P, :])
            xbf = pool.tile([P, D], BF16, tag=tag, name="xbf", bufs=2)
            nc.scalar.activation(xbf[:], x_nat[:], COPY)
            nc.sync.dma_start_transpose(xT_res[:, t], xbf[:])

        p1_cm = tc.tile_pool(name="p1", bufs=2)
        p1 = p1_cm.__enter__()
        try:
            # ------------ phase 1a: KV -> ctxU/Z accumulated in PSUM -----
            # start=True only clears the region a matmul writes, and PSUM
            # has_written bits persist across NEFF executions -- so the
            # accumulator must be explicitly zeroed (a zero-matmul writes 0
            # with has_written set), after which every ctx matmul
            # accumulates with start=False.
            acc_t = psum_ctx.tile([P, D], F32, tag="ctx", name="acc")
            acc = acc_t.rearrange("p (j k) -> p j k", k=P)
            for half in range(2):
                nc.tensor.matmul(
                    acc_t[:, half * 512:(half + 1) * 512],
                    lhsT=zero_bf[:], rhs=wkv_sb[:, 0, 0:512],
                    start=True, stop=True, skip_group_check=True)
            for t in range(NT):
                if not (prefetched and t < NPRE):
                    load_xT(t, p1)
                e_bf = p1.tile([P, H, DH], BF16, tag="E")
                v_bf = p1.tile([P, H, DH + 1], BF16, tag="V")
                for half in range(2):
                    kv_ps = psum_mm.tile([P, D], F32, tag="mm", name="kv_ps")
                    for nb in range(2):
                        col0 = half * D + nb * 512
                        for c in range(8):
                            nc.tensor.matmul(
                                kv_ps[:, nb * 512:(nb + 1) * 512],
                                lhsT=xT_res[:, t, c, :],
                                rhs=wkv_sb[:, c, col0:col0 + 512],
                                start=(c == 0), stop=(c == 7))
                    kv_v = kv_ps.rearrange("p (h t d) -> p h t d", t=2, d=DH)
                    hs = slice(half * 8, (half + 1) * 8)
                    nc.scalar.activation(e_bf[:, hs, :], kv_v[:, :, 0, :], EXP)
                    nc.vector.tensor_copy(out=v_bf[:, hs, 0:DH],
                                          in_=kv_v[:, :, 1, :])
                nc.vector.memset(v_bf[:, :, DH:DH + 1], 1.0)
                # 65-col outputs at 128-float pair stride: no bank crossing.
                # Only the first matmul into each PSUM bank clears it
                # (start=True); all others rely on per-element has_written.
                for j in range(NPAIR):
                    for odd in range(2):
                        h = 2 * j + odd
                        last = (t == NT - 1 and odd == 1 and j in (3, 7))
                        nc.tensor.matmul(
                            acc[odd * 64:(odd + 1) * 64, j, 0:65],
                            lhsT=e_bf[:, h, :],
                            rhs=v_bf[:, h, :],
                            start=False, stop=last,
                            tile_position=(0, odd * 64),
                            skip_group_check=True)
        finally:
            p1_cm.__exit__(None, None, None)

        # ------------- collective (8-way, Shared output) -------------
        # Each core owns one batch slot (one-hot bsel input); the other
        # slots are zero, so a full 8-way AllReduce -- the proven-stable
        # collective shape -- yields every pair's summed ctx in its slot.
        for b in range(B):
            ctx_loc = p2.tile([P, NPAIR, 65], F32, tag="ctxloc", bufs=2)
            nc.vector.tensor_tensor(
                ctx_loc[:],
                acc[:, :, 0:65],
                bsel_bc[:, b:b + 1, None].to_broadcast([P, NPAIR, 65]),
                MUL)
            nc.sync.dma_start(cc_in[:, b], ctx_loc[:])
        if no_collective:
            nc.sync.dma_start(cc_out[:], cc_in[:])
        else:
            nc.gpsimd.collective_compute(
                "AllReduce", mybir.AluOpType.add,
                replica_groups=[list(range(NCORES))],
                ins=[cc_in.opt()], outs=[cc_out.opt()])

        # phase-1b stages.  A+B1 (Q matmul, exp, softmax, qnT transpose) are
        # independent of the collective; B2 (y matmul) needs M.  The first
        # HOIST tiles run A+B1 before the M-build so the PE stays busy
        # through the collective.
        def q_stage_a(t):
            q_ps = psum_mm.tile([P, D], F32, tag="mm", name="q_ps")
            for nb in range(2):
                for c in range(8):
                    nc.tensor.matmul(
                        q_ps[:, nb * 512:(nb + 1) * 512],
                        lhsT=xT_res[:, t, c, :],
                        rhs=wq_sb[:, c, nb * 512:(nb + 1) * 512],
                        start=(c == 0), stop=(c == 7))
            eq = p2.tile([P, H, DH], BF16, tag="eq", name="eq")
            nc.scalar.activation(
                eq[:], q_ps.rearrange("p (h d) -> p h d", d=DH), EXP)
            return eq

        def q_stage_b1(t, eq):
            # row-softmax chain; the big elementwise multiply runs on the
            # otherwise-idle GpSimd engine and the PSUM evacuation on ACT so
            # the Vector engine never backs up the PE transposes
            rs = p2.tile([P, H], F32, tag="rs", name="rs")
            nc.vector.reduce_sum(rs[:], eq[:], axis=mybir.AxisListType.X)
            rsi = p2.tile([P, H], F32, tag="rsi", name="rsi")
            nc.vector.reciprocal_approx_fast(rsi[:], rs[:])
            rsb = p2.tile([P, H], BF16, tag="rsb", name="rsb")
            nc.vector.tensor_scalar_mul(rsb[:], rsi[:], S1 if Y_FP8 else 1.0)
            qn = p2.tile([P, D], BF16, tag="qn", name="qn")
            nc.vector.tensor_tensor(
                qn.rearrange("p (h d) -> p h d", d=DH),
                eq[:],
                rsb[:, :, None].to_broadcast([P, H, DH]),
                MUL)
            qnT = p2.tile([P, 8, P], FP8 if Y_FP8 else BF16, tag="qnT",
                          name="qnT", bufs=HOIST + 1)
            for g in range(2):
                tp2 = psum_tp.tile([P, 512], BF16, tag="tp", name="tp2")
                for k in range(4):
                    c = g * 4 + k
                    nc.tensor.transpose(tp2[:, k * P:(k + 1) * P],
                                        qn[:, c * P:(c + 1) * P], ident_bf)
                nc.scalar.activation(qnT[:, g * 4:(g + 1) * 4, :], tp2[:],
                                     COPY)
            if dbg is not None and t == 0:
                nc.sync.dma_start(dbg["eq"], eq[:])
                nc.sync.dma_start(dbg["qnT"], qnT[:])
            return qnT

        def q_stage_b2(t, qnT):
            y_ps = psum_ctx.tile([P, D], F32, tag="ctx", name="y_ps")
            if Y_FP8:
                for nb in range(2):
                    for c2 in range(4):
                        nc.tensor.matmul(
                            y_ps[:, nb * 512:(nb + 1) * 512],
                            lhsT=qnT[:, 2 * c2:2 * c2 + 2, :],
                            rhs=m_sb[:, 2 * c2:2 * c2 + 2,
                                     nb * 512:(nb + 1) * 512],
                            start=(c2 == 0), stop=(c2 == 3),
                            perf_mode=mybir.MatmulPerfMode.DoubleRow)
                y_sb = p2.tile([P, D], F32, tag="ysb", name="y_sb")
                nc.scalar.activation(y_sb[:], y_ps[:], COPY,
                                     scale=1.0 / (S1 * S2))
                nc.vector.tensor_add(out=y_sb[:], in0=y_sb[:], in1=blin_bc[:])
            else:
                for nb in range(2):
                    for c in range(8):
                        nc.tensor.matmul(
                            y_ps[:, nb * 512:(nb + 1) * 512],
                            lhsT=qnT[:, c, :],
                            rhs=m_sb[:, c, nb * 512:(nb + 1) * 512],
                            start=(c == 0), stop=(c == 7))
                y_sb = p2.tile([P, D], F32, tag="ysb", name="y_sb")
                nc.vector.tensor_add(out=y_sb[:], in0=y_ps[:], in1=blin_bc[:])
            nc.sync.dma_start(y_d[t * P:(t + 1) * P, :], y_sb[:])

        from collections import deque
        apend = deque()   # (t, eq) awaiting B1, lag 1 behind stage A
        pend = deque()    # (t, qnT) awaiting B2
        for t in range(HOIST):
            apend.append((t, q_stage_a(t)))
            if len(apend) > 1:
                ta, eqa = apend.popleft()
                pend.append((ta, q_stage_b1(ta, eqa)))

        # ---- mid: normalize ctx, build M = ctxn @ Wlin ----
        with ExitStack() as s2:
            mid = s2.enter_context(tc.tile_pool(name="mid", bufs=1))
            midw = s2.enter_context(tc.tile_pool(name="midw", bufs=2))
            wlin_sb = mid.tile([P, D // P, D], BF16, tag="wlin")
            for nb in range(2):
                for cc in range(0, 8, 2):
                    stage_cast(
                        wlin_sb[:, cc:cc + 2, nb * 512:(nb + 1) * 512],
                        wlin_src[:, cc:cc + 2, nb * 512:(nb + 1) * 512],
                        use_act=True)
            ctx_all = mid.tile([P, B, NPAIR, 65], F32, tag="ctxa")
            nc.sync.dma_start(ctx_all[:], cc_out[:])
            nc.vector.tensor_tensor(
                ctx_all[:], ctx_all[:],
                bsel_bc[:, :, None, None].to_broadcast([P, B, NPAIR, 65]),
                MUL)
            nc.vector.tensor_add(out=ctx_all[:, 0:2], in0=ctx_all[:, 0:2],
                                 in1=ctx_all[:, 2:4])
            nc.vector.tensor_add(out=ctx_all[:, 0], in0=ctx_all[:, 0],
                                 in1=ctx_all[:, 1])
            ctx_sb = ctx_all[:, 0]
            zinv = mid.tile([P, NPAIR], F32, tag="zinv")
            nc.vector.reciprocal_approx_fast(zinv[:], ctx_sb[:, :, 64])
            nc.vector.tensor_scalar_mul(zinv[:], zinv[:], SCALE)
            if dbg is not None:
                nc.sync.dma_start(dbg["ctx"], ctx_sb[:])

            def m_stage_a(j):
                ctxn = midw.tile([P, P], BF16, tag="ctxn", name="ctxn")
                nc.vector.memset(ctxn[:], 0.0)
                for odd in range(2):
                    o = odd * 64
                    nc.vector.tensor_scalar_mul(
                        ctxn[o:o + 64, o:o + 64],
                        ctx_sb[o:o + 64, j, 0:64],
                        zinv[o:o + 64, j:j + 1])
                tpp = psum_tp.tile([P, 512], BF16, tag="tp", name="tpp")
                nc.tensor.transpose(tpp[:, 0:P], ctxn[:], ident_bf)
                return tpp

            def m_stage_b(j, tpp):
                ctxnT = midw.tile([P, P], BF16, tag="ctxnT", name="ctxnT")
                nc.vector.tensor_copy(out=ctxnT[:], in_=tpp[:, 0:P])
                m_ps = psum_mm.tile([P, D], F32, tag="mm", name="m_ps")
                for nb in range(2):
                    nc.tensor.matmul(
                        m_ps[:, nb * 512:(nb + 1) * 512],
                        lhsT=ctxnT[:],
                        rhs=wlin_sb[:, j, nb * 512:(nb + 1) * 512],
                        start=True, stop=True)
                nc.scalar.activation(m_sb[:, j, :], m_ps[:], COPY,
                                     scale=S2 if Y_FP8 else 1.0)

            mpend = None
            for j in range(NPAIR):
                tpp = m_stage_a(j)
                if mpend is not None:
                    m_stage_b(*mpend)
                mpend = (j, tpp)
            m_stage_b(*mpend)

        if dbg is not None:
            nc.sync.dma_start(dbg["m"], m_sb[:])

        # ---- phase 1b steady state ----
        for t in range(NT):
            if t + HOIST < NT:
                apend.append((t + HOIST, q_stage_a(t + HOIST)))
            if apend:
                ta, eqa = apend.popleft()
                pend.append((ta, q_stage_b1(ta, eqa)))
            q_stage_b2(*pend.popleft())
            # next-rep prefetch: x tiles first (small, needed immediately at
            # the boundary), then the weight reload bulk behind them
            if load_next_weights is not None:
                if NT - 9 <= t < NT - 9 + NPRE:
                    # reuse the qn rotation (same shape/dtype) as staging so
                    # multi-rep builds need no extra SBUF
                    load_xT(t - (NT - 9), p2, tag="qn")
                elif t == NT - 6:
                    load_next_weights()
        assert not pend


_PROGRAM_CACHE = {}


def build_program(reps=1, debug_taps=False, single_core=False):
    key = (reps, debug_taps, single_core, Y_FP8)
    if key in _PROGRAM_CACHE:
        return _PROGRAM_CACHE[key]
    nc = bacc.Bacc("TRN2", target_bir_lowering=False, debug=False,
                   num_devices=1 if single_core else NCORES)
    x_d = nc.dram_tensor("x", [ROWS, D], F32, kind="ExternalInput").ap()
    wq_d = nc.dram_tensor("Wq", [D, D], F32, kind="ExternalInput").ap()
    wkv_d = nc.dram_tensor("Wkv", [D, 2 * D], F32, kind="ExternalInput").ap()
    wlin_d = nc.dram_tensor("Wlin", [D, D], F32, kind="ExternalInput").ap()
    blin_d = nc.dram_tensor("blin", [D], F32, kind="ExternalInput").ap()
    bsel_d = nc.dram_tensor("bsel", [B], F32, kind="ExternalInput").ap()
    y_d = nc.dram_tensor("y", [ROWS, D], F32, kind="ExternalOutput").ap()
    dbg = None
    if debug_taps:
        dbg = {
            "ctx": nc.dram_tensor("dbg_ctx", [P, NPAIR, 65], F32,
                                  kind="ExternalOutput").ap(),
            "m": nc.dram_tensor("dbg_m", [P, NPAIR, D],
                                FP8 if Y_FP8 else BF16,
                                kind="ExternalOutput").ap(),
            "eq": nc.dram_tensor("dbg_eq", [P, H, DH], BF16,
                                 kind="ExternalOutput").ap(),
            "qnT": nc.dram_tensor("dbg_qnT", [P, 8, P],
                                  FP8 if Y_FP8 else BF16,
                                  kind="ExternalOutput").ap(),
        }
    with tile.TileContext(nc) as tc:
        _emit(tc, nc, x_d, wq_d, wkv_d, wlin_d, blin_d, bsel_d, y_d, reps,
              dbg, no_collective=single_core)
    nc.compile()
    _PROGRAM_CACHE[key] = nc
    return nc


def make_in_maps(inputs):
    x = np.ascontiguousarray(inputs["x"], dtype=np.float32)
    wq = np.ascontiguousarray(inputs["Wq"], dtype=np.float32)
    wkv = np.ascontiguousarray(inputs["Wkv"], dtype=np.float32)
    wlin = np.ascontiguousarray(inputs["Wlin"], dtype=np.float32)
    blin = np.ascontiguousarray(inputs["blin"], dtype=np.float32)
    in_maps = []
    for c in range(NCORES):
        b, half = c // 2, c % 2
        x_shard = np.ascontiguousarray(
            x[b, half * S_HALF:(half + 1) * S_HALF, :])
        bsel = np.zeros(B, dtype=np.float32)
        bsel[b] = 1.0
        in_maps.append({"x": x_shard, "Wq": wq, "Wkv": wkv,
                        "Wlin": wlin, "blin": blin, "bsel": bsel})
    return in_maps


def kernel(**inputs) -> np.ndarray:
    nc = build_program(1)
    res = run_bass_kernel_spmd(nc, make_in_maps(inputs), list(range(NCORES)))
    y = np.empty((B, S, D), dtype=np.float32)
    for c in range(NCORES):
        b, half = c // 2, c % 2
        y[b, half * S_HALF:(half + 1) * S_HALF, :] = res.results[c]["y"]
    return y


if __name__ == "__main__":
    rng = np.random.default_rng(0)
    ins = {
        "x": rng.standard_normal((B, S, D), dtype=np.float32),
        "Wq": rng.standard_normal((D, D), dtype=np.float32) * 0.02,
        "Wkv": rng.standard_normal((D, 2 * D), dtype=np.float32) * 0.02,
        "Wlin": rng.standard_normal((D, D), dtype=np.float32) * 0.02,
        "blin": np.zeros(D, dtype=np.float32),
    }
    y = kernel(**ins)
    print("kernel output", y.shape, y.dtype, float(np.abs(y).mean()))


# revision 22
# speedup vs baseline: 1.2038x; 1.2038x over previous
"""Trainium2 Bass kernel for linear multi-head attention (v2).

Reference computation (B=4, S=8192, D=1024, H=16, DH=64):
    Q  = softmax((x@Wq) per-head over DH) * DH**-0.5
    K  = softmax((x@Wkv)[...,:DH] per-head over S)
    V  = (x@Wkv)[..., DH:]
    ctx = K^T @ V  per (b, h)               # [DH, DH]
    y  = (Q @ ctx  per head) @ Wlin + blin

Sharding v2: batch x seq-half. Core c handles batch b = c//2, sequence
half = c%2 -> 4096 rows per core, single local batch. The K-softmax
normalizer spans the full sequence, so each core accumulates the
unnormalized per-head context ctxU = sum_s exp(k_s) [v_s | 1] locally and
a PAIRWISE AllReduce (replica groups [2b, 2b+1]) sums the two halves --
4x less collective data and 4x fewer participants than seq-parallel x8.

Per-core pipeline (single pass over x; projection matmuls in bf16 with
fp32 accumulation):
  phase 1a (per 128-row tile): x tile -> cast bf16 -> PE-transpose ->
      xT kept RESIDENT in SBUF (8 MiB) -> KV = x@Wkv -> exp(K) ->
      per-head ctxU/Z single-shot matmuls accumulated IN PSUM across all
      32 tiles (per-element has_written handles first-write; only the
      first matmul per bank carries start=True)
  collective: pairwise AllReduce of [128, 8, 65] fp32 via DRAM bounce
  mid: ctxn = ctxU/Z * SCALE (bf16) -> PE-transpose -> M_j = ctxn_j @
      Wlin_j packed to M [128, 8, D] (bf16 or fp8).  Wlin is loaded
      fresh each rep here (it is only needed for 16 matmuls) to keep
      SBUF under budget; the DMA overlaps the collective.
  phase 1b (per tile, software-pipelined): Q = xT@Wq (resident xT, no
      reload/retranspose) -> exp -> row-normalize -> PE-transpose ->
      y = Qn @ M + blin -> store.  Stage A (Q matmul + exp) of the first
      HOIST tiles is emitted before the M-build so the PE never waits on
      the collective.
  The y matmul optionally runs in fp8-e4m3 DoubleRow (2x PE throughput):
      qn scaled by 2^6, M by 2^17, y unscaled by ACT copy with 2^-23.
  Wkv/Wq for rep i+1 are re-loaded during rep i's phase 1b so the rep
      boundary has no DMA stall (timing harness runs the body reps times).
"""

import sys

if "/opt/trn_rl_repo" not in sys.path:
    sys.path.insert(0, "/opt/trn_rl_repo")

from contextlib import ExitStack

import numpy as np

import concourse.bacc as bacc
import concourse.mybir as mybir
import concourse.tile as tile
from concourse.bass_utils import run_bass_kernel_spmd
from concourse.masks import make_identity

B, S, D = 4, 8192, 1024
H, DH = 16, 64
SCALE = DH ** -0.5
NCORES = 8
S_HALF = S // 2              # 4096 rows per core (one batch, half sequence)
ROWS = S_HALF
P = 128
NT = ROWS // P               # 32 tiles
NPAIR = H // 2               # 8 head pairs

F32 = mybir.dt.float32
BF16 = mybir.dt.bfloat16
FP8 = mybir.dt.float8e4
EXP = mybir.ActivationFunctionType.Exp
COPY = mybir.ActivationFunctionType.Copy
MUL = mybir.AluOpType.mult

Y_FP8 = True                 # y = Qn @ M in fp8-e4m3 DoubleRow
S1 = 64.0                    # qn scale (softmax values ~1/64)
S2 = float(2 ** 16)          # M scale (|M| ~ 3e-4, absmax ~2e-3; fp8e4
#                              max normal is 240 -> keep absmax*S2 < ~130)
HOIST = 16                   # phase-1b tiles run through qnT before M-build
NPRE = 3                     # next-rep x tiles prefetched during the tail


def _transpose_to(nc, psum_tp, dst, src, ident):
    """PE-transpose src [128, 1024] into dst [128, 8, 128] (feature-major)."""
    dt_ = src.dtype
    for g in range(2):
        tp = psum_tp.tile([P, 512], dt_, tag="tp")
        for k in range(4):
            c = g * 4 + k
            nc.tensor.transpose(tp[:, k * P:(k + 1) * P],
                                src[:, c * P:(c + 1) * P], ident)
        dst_v = dst[:, g * 4:(g + 1) * 4, :]
        if g == 0:
            nc.vector.tensor_copy(out=dst_v, in_=tp[:])
        else:
            nc.scalar.activation(dst_v, tp[:], COPY)


def _emit(tc, nc, x_d, wq_d, wkv_d, wlin_d, blin_d, bsel_d, y_d, reps,
          dbg=None,
          no_collective=False):
    with ExitStack() as top:
        const = top.enter_context(tc.tile_pool(name="const", bufs=1))
        dram = top.enter_context(tc.tile_pool(name="dram", bufs=1, space="DRAM"))
        # PSUM pools (8 banks exactly):
        #   psum_mm  [128,1024] x2 bufs = 4 banks: KV halves / Q / M-build
        #   psum_ctx [128,1024] x1 buf  = 2 banks: phase-1a ctx accumulator,
        #            reused as the y-matmul target in phase 1b
        #   psum_tp  [128,512]  x2 bufs = 2 banks: all PE transposes
        psum_mm = top.enter_context(tc.tile_pool(name="psum_mm", bufs=2, space="PSUM"))
        psum_ctx = top.enter_context(tc.tile_pool(name="psum_ctx", bufs=1, space="PSUM"))
        psum_tp = top.enter_context(tc.tile_pool(name="psum_tp", bufs=2, space="PSUM"))
        # long-lived SBUF: resident xT (8 MiB), Wkv/Wq (bf16), M
        resid = top.enter_context(tc.tile_pool(name="resid", bufs=1))
        work = top.enter_context(tc.tile_pool(name="work", bufs=2))
        xpool = top.enter_context(tc.tile_pool(name="xpool", bufs=3))

        ident_bf = const.tile([P, P], BF16, tag="ident_bf")
        make_identity(nc, ident_bf)
        blin_bc = const.tile([P, D], F32, tag="blin_bc")
        nc.sync.dma_start(blin_bc[:], blin_d[None, :].to_broadcast([P, D]))
        bsel_bc = const.tile([P, B], F32, tag="bsel_bc")
        nc.sync.dma_start(bsel_bc[:], bsel_d[None, :].to_broadcast([P, B]))
        zero_bf = const.tile([P, P], BF16, tag="zero_bf")
        nc.vector.memset(zero_bf[:], 0.0)

        wkv_sb = resid.tile([P, D // P, 2 * D], BF16, tag="wkv")
        wq_sb = resid.tile([P, D // P, D], BF16, tag="wq")
        xT_res = resid.tile([P, NT, D // P, P], BF16, tag="xTres")
        m_sb = resid.tile([P, NPAIR, D], FP8 if Y_FP8 else BF16, tag="M")

        wkv_src = wkv_d.rearrange("(c p) n -> p c n", p=P)
        wq_src = wq_d.rearrange("(c p) n -> p c n", p=P)
        wlin_src = wlin_d.rearrange("(c p) n -> p c n", p=P)

        def stage_cast(dst_slice, src_slice, use_act=False):
            wst = work.tile([P, 2, 512], F32, tag="wst", name="wst")
            nc.sync.dma_start(wst[:], src_slice)
            if use_act:
                nc.scalar.activation(dst_slice, wst[:], COPY)
            else:
                nc.vector.tensor_copy(out=dst_slice, in_=wst[:])

        def load_kvq():
            # wkv by (half, nb) column group so the first KV matmuls of the
            # next phase 1a wait on only 2 MiB of DMA
            for half in range(2):
                for nb in range(2):
                    col0 = half * D + nb * 512
                    for cc in range(0, 8, 2):
                        stage_cast(wkv_sb[:, cc:cc + 2, col0:col0 + 512],
                                   wkv_src[:, cc:cc + 2, col0:col0 + 512])
            for nb in range(2):
                for cc in range(0, 8, 2):
                    stage_cast(wq_sb[:, cc:cc + 2, nb * 512:(nb + 1) * 512],
                               wq_src[:, cc:cc + 2, nb * 512:(nb + 1) * 512])

        load_kvq()
        for rep in range(reps):
            _emit_once(tc, nc, x_d, y_d, dram,
                       (psum_mm, psum_ctx, psum_tp),
                       (wkv_sb, wq_sb, xT_res, m_sb),
                       wlin_src, stage_cast,
                       ident_bf, blin_bc, bsel_bc, zero_bf,
                       load_kvq if rep + 1 < reps else None,
                       xpool, prefetched=(rep > 0),
                       dbg=dbg, no_collective=no_collective)


def _emit_once(tc, nc, x_d, y_d, dram, psum, sbuf, wlin_src, stage_cast,
               ident_bf, blin_bc, bsel_bc, zero_bf, load_next_weights, xpool,
               prefetched=False, dbg=None, no_collective=False):
    psum_mm, psum_ctx, psum_tp = psum
    wkv_sb, wq_sb, xT_res, m_sb = sbuf
    cc_in = dram.tile([P, B, NPAIR, 65], F32, tag="cc_in")
    cc_out = dram.tile([P, B, NPAIR, 65], F32, tag="cc_out",
                       addr_space="Shared")

    # p2 outlives p1 (allocated first = disjoint zone); p1 closes after
    # phase 1a so the mid pools reuse its SBUF space.
    with ExitStack() as s1:
        p2 = s1.enter_context(tc.tile_pool(name="p2", bufs=2))

        def load_xT(t, pool, tag="xb"):
            x_nat = xpool.tile([P, D], F32, tag="x", name="x_nat")
            nc.sync.dma_start(x_nat[:], x_d[t * P:(t + 1) * P, :])
            xbf = pool.tile([P, D], BF16, tag=tag, name="xbf", bufs=2)
            nc.scalar.activation(xbf[:], x_nat[:], COPY)
            _transpose_to(nc, psum_tp, xT_res[:, t], xbf, ident_bf)

        p1_cm = tc.tile_pool(name="p1", bufs=2)
        p1 = p1_cm.__enter__()
        try:
            # ------------ phase 1a: KV -> ctxU/Z accumulated in PSUM -----
            # start=True only clears the region a matmul writes, and PSUM
            # has_written bits persist across NEFF executions -- so the
            # accumulator must be explicitly zeroed (a zero-matmul writes 0
            # with has_written set), after which every ctx matmul
            # accumulates with start=False.
            acc_t = psum_ctx.tile([P, D], F32, tag="ctx", name="acc")
            acc = acc_t.rearrange("p (j k) -> p j k", k=P)
            for half in range(2):
                nc.tensor.matmul(
                    acc_t[:, half * 512:(half + 1) * 512],
                    lhsT=zero_bf[:], rhs=wkv_sb[:, 0, 0:512],
                    start=True, stop=True, skip_group_check=True)
            for t in range(NT):
                if not (prefetched and t < NPRE):
                    load_xT(t, p1)
                e_bf = p1.tile([P, H, DH], BF16, tag="E")
                v_bf = p1.tile([P, H, DH + 1], BF16, tag="V")
                for half in range(2):
                    kv_ps = psum_mm.tile([P, D], F32, tag="mm", name="kv_ps")
                    for nb in range(2):
                        col0 = half * D + nb * 512
                        for c in range(8):
                            nc.tensor.matmul(
                                kv_ps[:, nb * 512:(nb + 1) * 512],
                                lhsT=xT_res[:, t, c, :],
                                rhs=wkv_sb[:, c, col0:col0 + 512],
                                start=(c == 0), stop=(c == 7))
                    kv_v = kv_ps.rearrange("p (h t d) -> p h t d", t=2, d=DH)
                    hs = slice(half * 8, (half + 1) * 8)
                    nc.scalar.activation(e_bf[:, hs, :], kv_v[:, :, 0, :], EXP)
                    nc.vector.tensor_copy(out=v_bf[:, hs, 0:DH],
                                          in_=kv_v[:, :, 1, :])
                nc.vector.memset(v_bf[:, :, DH:DH + 1], 1.0)
                # 65-col outputs at 128-float pair stride: no bank crossing.
                # Only the first matmul into each PSUM bank clears it
                # (start=True); all others rely on per-element has_written.
                for j in range(NPAIR):
                    for odd in range(2):
                        h = 2 * j + odd
                        last = (t == NT - 1 and odd == 1 and j in (3, 7))
                        nc.tensor.matmul(
                            acc[odd * 64:(odd + 1) * 64, j, 0:65],
                            lhsT=e_bf[:, h, :],
                            rhs=v_bf[:, h, :],
                            start=False, stop=last,
                            tile_position=(0, odd * 64),
                            skip_group_check=True)
        finally:
            p1_cm.__exit__(None, None, None)

        # ------------- collective (8-way, Shared output) -------------
        # Each core owns one batch slot (one-hot bsel input); the other
        # slots are zero, so a full 8-way AllReduce -- the proven-stable
        # collective shape -- yields every pair's summed ctx in its slot.
        for b in range(B):
            ctx_loc = p2.tile([P, NPAIR, 65], F32, tag="ctxloc", bufs=2)
            nc.vector.tensor_tensor(
                ctx_loc[:],
                acc[:, :, 0:65],
                bsel_bc[:, b:b + 1, None].to_broadcast([P, NPAIR, 65]),
                MUL)
            nc.sync.dma_start(cc_in[:, b], ctx_loc[:])
        if no_collective:
            nc.sync.dma_start(cc_out[:], cc_in[:])
        else:
            nc.gpsimd.collective_compute(
                "AllReduce", mybir.AluOpType.add,
                replica_groups=[list(range(NCORES))],
                ins=[cc_in.opt()], outs=[cc_out.opt()])

        # phase-1b stages.  A+B1 (Q matmul, exp, softmax, qnT transpose) are
        # independent of the collective; B2 (y matmul) needs M.  The first
        # HOIST tiles run A+B1 before the M-build so the PE stays busy
        # through the collective.
        def q_stage_a(t):
            q_ps = psum_mm.tile([P, D], F32, tag="mm", name="q_ps")
            for nb in range(2):
                for c in range(8):
                    nc.tensor.matmul(
                        q_ps[:, nb * 512:(nb + 1) * 512],
                        lhsT=xT_res[:, t, c, :],
                        rhs=wq_sb[:, c, nb * 512:(nb + 1) * 512],
                        start=(c == 0), stop=(c == 7))
            eq = p2.tile([P, H, DH], BF16, tag="eq", name="eq")
            nc.scalar.activation(
                eq[:], q_ps.rearrange("p (h d) -> p h d", d=DH), EXP)
            return eq

        def q_stage_b1(t, eq):
            # row-softmax chain; the big elementwise multiply runs on the
            # otherwise-idle GpSimd engine and the PSUM evacuation on ACT so
            # the Vector engine never backs up the PE transposes
            rs = p2.tile([P, H], F32, tag="rs", name="rs")
            nc.vector.reduce_sum(rs[:], eq[:], axis=mybir.AxisListType.X)
            rsi = p2.tile([P, H], F32, tag="rsi", name="rsi")
            nc.vector.reciprocal_approx_fast(rsi[:], rs[:])
            rsb = p2.tile([P, H], BF16, tag="rsb", name="rsb")
            nc.vector.tensor_scalar_mul(rsb[:], rsi[:], S1 if Y_FP8 else 1.0)
            qn = p2.tile([P, D], BF16, tag="qn", name="qn")
            nc.vector.tensor_tensor(
                qn.rearrange("p (h d) -> p h d", d=DH),
                eq[:],
                rsb[:, :, None].to_broadcast([P, H, DH]),
                MUL)
            qnT = p2.tile([P, 8, P], FP8 if Y_FP8 else BF16, tag="qnT",
                          name="qnT", bufs=HOIST + 1)
            for g in range(2):
                tp2 = psum_tp.tile([P, 512], BF16, tag="tp", name="tp2")
                for k in range(4):
                    c = g * 4 + k
                    nc.tensor.transpose(tp2[:, k * P:(k + 1) * P],
                                        qn[:, c * P:(c + 1) * P], ident_bf)
                nc.scalar.activation(qnT[:, g * 4:(g + 1) * 4, :], tp2[:],
                                     COPY)
            if dbg is not None and t == 0:
                nc.sync.dma_start(dbg["eq"], eq[:])
                nc.sync.dma_start(dbg["qnT"], qnT[:])
            return qnT

        def q_stage_b2(t, qnT):
            y_ps = psum_ctx.tile([P, D], F32, tag="ctx", name="y_ps")
            if Y_FP8:
                for nb in range(2):
                    for c2 in range(4):
                        nc.tensor.matmul(
                            y_ps[:, nb * 512:(nb + 1) * 512],
                            lhsT=qnT[:, 2 * c2:2 * c2 + 2, :],
                            rhs=m_sb[:, 2 * c2:2 * c2 + 2,
                                     nb * 512:(nb + 1) * 512],
                            start=(c2 == 0), stop=(c2 == 3),
                            perf_mode=mybir.MatmulPerfMode.DoubleRow)
                y_sb = p2.tile([P, D], F32, tag="ysb", name="y_sb")
                nc.scalar.activation(y_sb[:], y_ps[:], COPY,
                                     scale=1.0 / (S1 * S2))
                nc.vector.tensor_add(out=y_sb[:], in0=y_sb[:], in1=blin_bc[:])
            else:
                for nb in range(2):
                    for c in range(8):
                        nc.tensor.matmul(
                            y_ps[:, nb * 512:(nb + 1) * 512],
                            lhsT=qnT[:, c, :],
                            rhs=m_sb[:, c, nb * 512:(nb + 1) * 512],
                            start=(c == 0), stop=(c == 7))
                y_sb = p2.tile([P, D], F32, tag="ysb", name="y_sb")
                nc.vector.tensor_add(out=y_sb[:], in0=y_ps[:], in1=blin_bc[:])
            nc.sync.dma_start(y_d[t * P:(t + 1) * P, :], y_sb[:])

        from collections import deque
        apend = deque()   # (t, eq) awaiting B1, lag 1 behind stage A
        pend = deque()    # (t, qnT) awaiting B2
        for t in range(HOIST):
            apend.append((t, q_stage_a(t)))
            if len(apend) > 1:
                ta, eqa = apend.popleft()
                pend.append((ta, q_stage_b1(ta, eqa)))

        # ---- mid: normalize ctx, build M = ctxn @ Wlin ----
        with ExitStack() as s2:
            mid = s2.enter_context(tc.tile_pool(name="mid", bufs=1))
            midw = s2.enter_context(tc.tile_pool(name="midw", bufs=2))
            wlin_sb = mid.tile([P, D // P, D], BF16, tag="wlin")
            for nb in range(2):
                for cc in range(0, 8, 2):
                    stage_cast(
                        wlin_sb[:, cc:cc + 2, nb * 512:(nb + 1) * 512],
                        wlin_src[:, cc:cc + 2, nb * 512:(nb + 1) * 512],
                        use_act=True)
            ctx_all = mid.tile([P, B, NPAIR, 65], F32, tag="ctxa")
            nc.sync.dma_start(ctx_all[:], cc_out[:])
            nc.vector.tensor_tensor(
                ctx_all[:], ctx_all[:],
                bsel_bc[:, :, None, None].to_broadcast([P, B, NPAIR, 65]),
                MUL)
            nc.vector.tensor_add(out=ctx_all[:, 0:2], in0=ctx_all[:, 0:2],
                                 in1=ctx_all[:, 2:4])
            nc.vector.tensor_add(out=ctx_all[:, 0], in0=ctx_all[:, 0],
                                 in1=ctx_all[:, 1])
            ctx_sb = ctx_all[:, 0]
            zinv = mid.tile([P, NPAIR], F32, tag="zinv")
            nc.vector.reciprocal_approx_fast(zinv[:], ctx_sb[:, :, 64])
            nc.vector.tensor_scalar_mul(zinv[:], zinv[:], SCALE)
            if dbg is not None:
                nc.sync.dma_start(dbg["ctx"], ctx_sb[:])

            def m_stage_a(j):
                ctxn = midw.tile([P, P], BF16, tag="ctxn", name="ctxn")
                nc.vector.memset(ctxn[:], 0.0)
                for odd in range(2):
                    o = odd * 64
                    nc.vector.tensor_scalar_mul(
                        ctxn[o:o + 64, o:o + 64],
                        ctx_sb[o:o + 64, j, 0:64],
                        zinv[o:o + 64, j:j + 1])
                tpp = psum_tp.tile([P, 512], BF16, tag="tp", name="tpp")
                nc.tensor.transpose(tpp[:, 0:P], ctxn[:], ident_bf)
                return tpp

            def m_stage_b(j, tpp):
                ctxnT = midw.tile([P, P], BF16, tag="ctxnT", name="ctxnT")
                nc.vector.tensor_copy(out=ctxnT[:], in_=tpp[:, 0:P])
                m_ps = psum_mm.tile([P, D], F32, tag="mm", name="m_ps")
                for nb in range(2):
                    nc.tensor.matmul(
                        m_ps[:, nb * 512:(nb + 1) * 512],
                        lhsT=ctxnT[:],
                        rhs=wlin_sb[:, j, nb * 512:(nb + 1) * 512],
                        start=True, stop=True)
                nc.scalar.activation(m_sb[:, j, :], m_ps[:], COPY,
                                     scale=S2 if Y_FP8 else 1.0)

            mpend = None
            for j in range(NPAIR):
                tpp = m_stage_a(j)
                if mpend is not None:
                    m_stage_b(*mpend)
                mpend = (j, tpp)
            m_stage_b(*mpend)

        if dbg is not None:
            nc.sync.dma_start(dbg["m"], m_sb[:])

        # ---- phase 1b steady state ----
        for t in range(NT):
            if t + HOIST < NT:
                apend.append((t + HOIST, q_stage_a(t + HOIST)))
            if apend:
                ta, eqa = apend.popleft()
                pend.append((ta, q_stage_b1(ta, eqa)))
            q_stage_b2(*pend.popleft())
            # next-rep prefetch: x tiles first (small, needed immediately at
            # the boundary), then the weight reload bulk behind them
            if load_next_weights is not None:
                if NT - 9 <= t < NT - 9 + NPRE:
                    # reuse the qn rotation (same shape/dtype) as staging so
                    # multi-rep builds need no extra SBUF
                    load_xT(t - (NT - 9), p2, tag="qn")
                elif t == NT - 6:
                    load_next_weights()
        assert not pend


_PROGRAM_CACHE = {}


def build_program(reps=1, debug_taps=False, single_core=False):
    key = (reps, debug_taps, single_core, Y_FP8)
    if key in _PROGRAM_CACHE:
        return _PROGRAM_CACHE[key]
    nc = bacc.Bacc("TRN2", target_bir_lowering=False, debug=False,
                   num_devices=1 if single_core else NCORES)
    x_d = nc.dram_tensor("x", [ROWS, D], F32, kind="ExternalInput").ap()
    wq_d = nc.dram_tensor("Wq", [D, D], F32, kind="ExternalInput").ap()
    wkv_d = nc.dram_tensor("Wkv", [D, 2 * D], F32, kind="ExternalInput").ap()
    wlin_d = nc.dram_tensor("Wlin", [D, D], F32, kind="ExternalInput").ap()
    blin_d = nc.dram_tensor("blin", [D], F32, kind="ExternalInput").ap()
    bsel_d = nc.dram_tensor("bsel", [B], F32, kind="ExternalInput").ap()
    y_d = nc.dram_tensor("y", [ROWS, D], F32, kind="ExternalOutput").ap()
    dbg = None
    if debug_taps:
        dbg = {
            "ctx": nc.dram_tensor("dbg_ctx", [P, NPAIR, 65], F32,
                                  kind="ExternalOutput").ap(),
            "m": nc.dram_tensor("dbg_m", [P, NPAIR, D],
                                FP8 if Y_FP8 else BF16,
                                kind="ExternalOutput").ap(),
            "eq": nc.dram_tensor("dbg_eq", [P, H, DH], BF16,
                                 kind="ExternalOutput").ap(),
            "qnT": nc.dram_tensor("dbg_qnT", [P, 8, P],
                                  FP8 if Y_FP8 else BF16,
                                  kind="ExternalOutput").ap(),
        }
    with tile.TileContext(nc) as tc:
        _emit(tc, nc, x_d, wq_d, wkv_d, wlin_d, blin_d, bsel_d, y_d, reps,
              dbg, no_collective=single_core)
    nc.compile()
    _PROGRAM_CACHE[key] = nc
    return nc


def make_in_maps(inputs):
    x = np.ascontiguousarray(inputs["x"], dtype=np.float32)
    wq = np.ascontiguousarray(inputs["Wq"], dtype=np.float32)
    wkv = np.ascontiguousarray(inputs["Wkv"], dtype=np.float32)
    wlin = np.ascontiguousarray(inputs["Wlin"], dtype=np.float32)
    blin = np.ascontiguousarray(inputs["blin"], dtype=np.float32)
    in_maps = []
    for c in range(NCORES):
        b, half = c // 2, c % 2
        x_shard = np.ascontiguousarray(
            x[b, half * S_HALF:(half + 1) * S_HALF, :])
        bsel = np.zeros(B, dtype=np.float32)
        bsel[b] = 1.0
        in_maps.append({"x": x_shard, "Wq": wq, "Wkv": wkv,
                        "Wlin": wlin, "blin": blin, "bsel": bsel})
    return in_maps


def kernel(**inputs) -> np.ndarray:
    nc = build_program(1)
    res = run_bass_kernel_spmd(nc, make_in_maps(inputs), list(range(NCORES)))
    y = np.empty((B, S, D), dtype=np.float32)
    for c in range(NCORES):
        b, half = c // 2, c % 2
        y[b, half * S_HALF:(half + 1) * S_HALF, :] = res.results[c]["y"]
    return y


if __name__ == "__main__":
    rng = np.random.default_rng(0)
    ins = {
        "x": rng.standard_normal((B, S, D), dtype=np.float32),
        "Wq": rng.standard_normal((D, D), dtype=np.float32) * 0.02,
        "Wkv": rng.standard_normal((D, 2 * D), dtype=np.float32) * 0.02,
        "Wlin": rng.standard_normal((D, D), dtype=np.float32) * 0.02,
        "blin": np.zeros(D, dtype=np.float32),
    }
    y = kernel(**ins)
    print("kernel output", y.shape, y.dtype, float(np.abs(y).mean()))
